# revision 64
# baseline (speedup 1.0000x reference)
"""PVT-style spatial-reduction attention on 8 TRN2 NeuronCores (Bass/Tile).

Strategy: data-parallel over batch (16 images -> 2 per core). Each core runs an
identical single-core program on its shard; no collectives.

Host-side prep (inside kernel(), part of sharding/layout):
  - x transposed to channel-major xT [2, 256, 4096] so matmul operands need no
    on-device transposition of the big activation.
  - attention scale hd^-0.5 folded into Wq; LayerNorm gamma/beta folded into
    Wkv algebraically (exact); conv weights pre-transposed per tap.

Device pipeline per batch (all matmuls float32r):
  qT = Wq^T @ xT                       (feature-major q)
  xr = sum over 16 conv taps of gathered-xT^T @ w_tap   (strided-gather lhsT)
  LN over free dim (quake rsqrt on DVE), PE-transpose of x_norm (small)
  kT = Wk^T @ xnT ; v = xn @ Wv
  per 512-row block, per head:
    S^T = kT_h^T @ qT_h  (keys on partitions)  -> exp on ScalarE (PSUM->SBUF)
    sums = ones32^T @ P^T  (32-replicated row sums via matmul)
    O^T  = v_h^T @ P^T     (unnormalized)
    R = reciprocal_approx_fast(sums); O-norm fused into PSUM->SBUF move
  out = O_norm^T^T @ Wp (+bp via K=1 matmul)  -> natural layout -> DMA out
"""

import os
import sys
from contextlib import ExitStack

if "/opt/trn_rl_repo" not in sys.path:
    sys.path.insert(0, "/opt/trn_rl_repo")

import numpy as np
import ml_dtypes

import concourse.bass as bass
import concourse.bacc as bacc
import concourse.tile as tile
from concourse import mybir
from concourse.bass_utils import run_bass_kernel_spmd

N_CORES = 8
B, N, C = 16, 4096, 256
B_LOC = B // N_CORES
H8, HD, M = 8, 32, 256
NBLK, BLK = 8, 512
F32 = mybir.dt.float32
F32R = mybir.dt.float32r
BF16 = mybir.dt.bfloat16
I32 = mybir.dt.int32
AF = mybir.ActivationFunctionType
OP = mybir.AluOpType
AX = mybir.AxisListType

KERNEL_STATS = {}


def _r(ap):
    return ap


def _kernel_body(ctx, tc, out, ins, with_bp):
    nc = tc.nc
    (xT_d, wqkvp_d, srw_d, f32p_d) = ins

    consts = ctx.enter_context(tc.tile_pool(name="consts", bufs=1))
    sb_xT = ctx.enter_context(tc.tile_pool(name="sb_xT", bufs=2))
    sb_qT = ctx.enter_context(tc.tile_pool(name="sb_qT", bufs=2))
    sb_oT = ctx.enter_context(tc.tile_pool(name="sb_oT", bufs=1))
    sb_pT = ctx.enter_context(tc.tile_pool(name="sb_pT", bufs=12))
    sb_kv = ctx.enter_context(tc.tile_pool(name="sb_kv", bufs=2))
    sb_ln = ctx.enter_context(tc.tile_pool(name="sb_ln", bufs=2))
    sb_R = ctx.enter_context(tc.tile_pool(name="sb_R", bufs=4))
    sb_st = ctx.enter_context(tc.tile_pool(name="sb_st", bufs=4))
    ps_s = ctx.enter_context(tc.tile_pool(name="ps_s", bufs=2, space="PSUM"))
    ps_att = ctx.enter_context(tc.tile_pool(name="ps_att", bufs=2, space="PSUM"))
    ps_gen = ctx.enter_context(tc.tile_pool(name="ps_gen", bufs=2, space="PSUM"))

    cst = {}

    def cload(name, src, shape, dtype=F32):
        t = consts.tile(shape, dtype, tag=name, name=name)
        nc.sync.dma_start(t[:], src)
        return t

    def emit_consts():
        # One packed bf16 weight blob (scalar-engine DMA queue) and one
        # packed f32 blob; ones32 via memset. Keeps the Sync queue free for
        # xT chunks during startup.
        wb = consts.tile([128, 2048], BF16, tag="wb", name="wb")
        nc.scalar.dma_start(wb[:], wqkvp_d[:, :])
        cst["wq"] = [wb[:, 256 * k:256 * (k + 1)] for k in range(2)]
        cst["wk"] = [wb[:, 512 + 256 * k:512 + 256 * (k + 1)]
                     for k in range(2)]
        cst["wv"] = [wb[:, 1024 + 256 * k:1024 + 256 * (k + 1)]
                     for k in range(2)]
        cst["wp"] = [wb[:, 1536 + 256 * k:1536 + 256 * (k + 1)]
                     for k in range(2)]
        fb = consts.tile([128, 1154], F32, tag="fb", name="fb")
        nc.scalar.dma_start(fb[:], f32p_d[:, :])
        cst["srb"] = fb[:, 0:256]
        cst["bv"] = fb[:, 256:512]
        cst["bpt"] = fb[:, 512:1024]
        cst["eye"] = fb[:, 1024:1152]
        cst["bk"] = [fb[:, 1152 + k:1153 + k] for k in range(2)]
        ones_t = consts.tile([128, 32], BF16, tag="ones32", name="ones32")
        nc.gpsimd.memset(ones_t[:], 1.0)
        cst["ones32"] = ones_t
        magic_t = consts.tile([128, 1], I32, tag="magic", name="magic")
        nc.gpsimd.memset(magic_t[:], 0x5F3759DF)
        cst["magic"] = magic_t
        c15_t = consts.tile([128, 1], F32, tag="c15", name="c15")
        nc.gpsimd.memset(c15_t[:], 1.5)
        cst["c15"] = c15_t

    # Per-batch state carried across chunks
    S = [dict() for _ in range(B_LOC)]

    def chunk_load_x(b):
        s = S[b]
        s["xT"] = sb_xT.tile([128, 2 * N], BF16, tag="xT", name=f"xt{b}")
        nc.sync.dma_start(s["xT"][:], xT_d[b])

    def _xs(b, ki, c0, c1):
        return S[b]["xT"][:, N * ki + c0:N * ki + c1]

    def _conv_mo(b, mo):
        psc = ps_gen.tile([128, C], F32, tag="g", name=f"psc{b}{mo}")
        for tap in range(16):
            for ki in range(2):
                nc.tensor.matmul(
                    psc[:],
                    _r(_xs(b, ki, 256 * tap + 128 * mo,
                           256 * tap + 128 * (mo + 1))),
                    _r(cst["srw"][:, 512 * tap + C * ki:
                                  512 * tap + C * (ki + 1)]),
                    start=(tap == 0 and ki == 0),
                    stop=(tap == 15 and ki == 1),
                )
        return psc

    def chunk_conv0(b):
        S[b]["psc0"] = _conv_mo(b, 0)

    def startup_b0():
        """Batch-0 warm-up: interleave xT DMA chunks, srw loads (vector DMA
        queue) and conv-tap matmuls so the PE starts as soon as data lands.
        Weight consts go on the scalar DMA queue in parallel."""
        s = S[0]
        s["xT"] = sb_xT.tile([128, 2 * N], BF16, tag="xT", name="xt0")
        xv = s["xT"].rearrange("p (k n) -> p k n", k=2)
        xsrc = xT_d[0].rearrange("p (k n) -> p k n", k=2)
        psc = [ps_gen.tile([128, C], F32, tag="g", name=f"psc0{mo}")
               for mo in range(2)]
        srw_t = consts.tile([128, 8192], BF16, tag="srw", name="srw")
        cst["srw"] = srw_t
        for q4 in range(4):
            if q4 % 2 == 0:
                h = q4 // 2
                nc.gpsimd.dma_start(srw_t[:, 4096 * h:4096 * (h + 1)],
                                    srw_d[:, 4096 * h:4096 * (h + 1)])
            nc.sync.dma_start(xv[:, :, 1024 * q4:1024 * (q4 + 1)],
                              xsrc[:, :, 1024 * q4:1024 * (q4 + 1)])
            if q4 == 0:
                emit_consts()
            for mo in range(2):
                for tap in range(4 * q4, 4 * q4 + 4):
                    for ki in range(2):
                        nc.tensor.matmul(
                            psc[mo][:],
                            _r(_xs(0, ki, 256 * tap + 128 * mo,
                                   256 * tap + 128 * (mo + 1))),
                            _r(cst["srw"][:, 512 * tap + C * ki:
                                          512 * tap + C * (ki + 1)]),
                            start=(tap == 0 and ki == 0),
                            stop=(tap == 15 and ki == 1),
                        )
        # Interleave q-block matmuls (on the startup-idle ps_att pool) with
        # the LN chains so the PE never waits on the DVE here.
        s["xn0"] = _ln(0, 0, psc[0])
        _q_blocks(0, range(0, 4), pool=ps_att, tag="att")
        s["xn1"] = _ln(0, 1, psc[1])
        _q_blocks(0, range(4, 8), pool=ps_att, tag="att")

    def _ln(b, mo, psc):
        s = S[b]
        xb = sb_ln.tile([128, C], F32, tag="xb", name=f"xb{b}{mo}")
        nc.vector.tensor_add(xb[:], psc[:], cst["srb"][:])
        ssum = sb_ln.tile([128, 1], F32, tag="ssum", name=f"ssum{b}{mo}")
        nc.vector.tensor_reduce(ssum[:], xb[:], axis=AX.X, op=OP.add)
        mu = sb_ln.tile([128, 1], F32, tag="mu", name=f"mu{b}{mo}")
        nc.vector.tensor_scalar_mul(mu[:], ssum[:], 1.0 / C)
        xc = sb_ln.tile([128, C], F32, tag="xc", name=f"xc{b}{mo}")
        nc.vector.tensor_scalar_sub(xc[:], xb[:], mu[:, 0:1])
        sq = sb_ln.tile([128, C], F32, tag="sq", name=f"sq{b}{mo}")
        vraw = sb_ln.tile([128, 1], F32, tag="vraw", name=f"vraw{b}{mo}")
        nc.vector.scalar_tensor_tensor(
            sq[:], xc[:], 0.0, xc[:], op0=OP.add, op1=OP.mult,
            accum_out=vraw[:, 0:1])
        veps = sb_ln.tile([128, 1], F32, tag="veps", name=f"veps{b}{mo}")
        nc.vector.tensor_scalar(veps[:], vraw[:], 1.0 / C, 1e-5,
                                op0=OP.mult, op1=OP.add)
        vh = sb_ln.tile([128, 1], F32, tag="vh", name=f"vh{b}{mo}")
        nc.vector.tensor_scalar_mul(vh[:], veps[:], -0.5)
        sh = sb_ln.tile([128, 1], I32, tag="sh", name=f"sh{b}{mo}")
        nc.vector.tensor_scalar(sh[:], veps[:].bitcast(I32), 1, None,
                                op0=OP.logical_shift_right)
        y = sb_ln.tile([128, 1], F32, tag="y", name=f"y{b}{mo}")
        nc.vector.scalar_tensor_tensor(
            y[:].bitcast(I32), cst["magic"][:], 0, sh[:],
            op0=OP.bypass, op1=OP.subtract)
        for it in range(3):
            yy = sb_ln.tile([128, 1], F32, tag=f"yy{it}", name=f"yy{b}{mo}{it}")
            nc.vector.tensor_mul(yy[:], y[:], y[:])
            t2 = sb_ln.tile([128, 1], F32, tag=f"t2{it}", name=f"t2{b}{mo}{it}")
            nc.vector.scalar_tensor_tensor(
                t2[:], yy[:], vh[:, 0:1], cst["c15"][:],
                op0=OP.mult, op1=OP.add)
            y2 = sb_ln.tile([128, 1], F32, tag=f"y2{it}", name=f"yn{b}{mo}{it}")
            nc.vector.tensor_mul(y2[:], y[:], t2[:])
            y = y2
        xn = sb_ln.tile([128, C], F32, tag="xn", name=f"xn{b}{mo}")
        nc.vector.tensor_scalar_mul(xn[:], xc[:], y[:, 0:1])
        return xn

    def chunk_conv1_ln0(b):
        s = S[b]
        s["psc1"] = _conv_mo(b, 1)
        s["xn0"] = _ln(b, 0, s["psc0"])

    def chunk_kv(b):
        s = S[b]
        if "xn1" not in s:
            s["xn1"] = _ln(b, 1, s["psc1"])
        xn_sb = [s["xn0"], s["xn1"]]
        xnT_sb = []
        for i in range(2):
            xnT = sb_kv.tile([128, M], BF16, tag=f"xnT{i}", name=f"xnT{b}{i}")
            xnT_sb.append(xnT)
        for i in range(2):
            for j in range(2):
                ps_t = ps_gen.tile([128, 128], F32, tag="g",
                                   name=f"pst{b}{i}{j}")
                nc.tensor.transpose(ps_t[:],
                                    xn_sb[j][:, 128 * i:128 * (i + 1)],
                                    cst["eye"][:])
                nc.vector.tensor_copy(xnT_sb[i][:, 128 * j:128 * (j + 1)],
                                      ps_t[:])
        kT_sb, v_sb = [], []
        for mo in range(2):
            ps_k = ps_gen.tile([128, M], F32, tag="g", name=f"psk{b}{mo}")
            for ki in range(2):
                nc.tensor.matmul(
                    ps_k[:], _r(cst["wk"][ki][:, 128 * mo:128 * (mo + 1)]),
                    _r(xnT_sb[ki][:]), start=(ki == 0), stop=(ki == 1))
            kT = sb_kv.tile([128, M], BF16, tag=f"kT{mo}", name=f"kT{b}{mo}")
            nc.vector.tensor_scalar_add(kT[:], ps_k[:], cst["bk"][mo][:, 0:1])
            kT_sb.append(kT)
        for mo in range(2):
            ps_v = ps_gen.tile([128, C], F32, tag="g", name=f"psv{b}{mo}")
            for ki in range(2):
                nc.tensor.matmul(
                    ps_v[:], _r(xnT_sb[ki][:, 128 * mo:128 * (mo + 1)]),
                    _r(cst["wv"][ki][:]), start=(ki == 0), stop=(ki == 1))
            v = sb_kv.tile([128, C], BF16, tag=f"v{mo}", name=f"v{b}{mo}")
            nc.vector.tensor_add(v[:], ps_v[:], cst["bv"][:])
            v_sb.append(v)
        s["kT"] = kT_sb
        s["v"] = v_sb

    def _q_blocks(b, blks, pool=None, tag="g"):
        s = S[b]
        pool = pool or ps_gen
        if "qT" not in s:
            s["qT"] = [sb_qT.tile([128, N], BF16, tag=f"qT{k}", bufs=2,
                                  name=f"qT{b}{k}") for k in range(2)]
        for blk in blks:
            for mo in range(2):
                ps_q = pool.tile([128, BLK], F32, tag=tag,
                                 name=f"psq{b}{blk}{mo}")
                for ki in range(2):
                    nc.tensor.matmul(
                        ps_q[:], _r(cst["wq"][ki][:, 128 * mo:128 * (mo + 1)]),
                        _r(_xs(b, ki, BLK * blk, BLK * (blk + 1))),
                        start=(ki == 0), stop=(ki == 1))
                nc.vector.tensor_copy(
                    s["qT"][mo][:, BLK * blk:BLK * (blk + 1)], ps_q[:])

    A_CHUNKS = [
        chunk_load_x,
        chunk_conv0,
        chunk_conv1_ln0,
        lambda b: _q_blocks(b, range(0, 4)),
        lambda b: _q_blocks(b, range(4, 8)),
        chunk_kv,
    ]

    def emit_proj(b, pblk, tagx):
        s = S[b]
        oT_sb = s["oT"]
        for rbp in range(2):
            ps_pj = ps_gen.tile([128, BLK], F32, tag="g",
                                name=f"pspj{tagx}{b}{pblk}{rbp}")
            for half in range(2):
                rb = 4 * pblk + 2 * rbp + half
                for ki in range(2):
                    nc.tensor.matmul(
                        ps_pj[:, C * half:C * (half + 1)],
                        _r(oT_sb[ki][:, 128 * rb:128 * (rb + 1)]),
                        _r(cst["wp"][ki][:]),
                        start=(ki == 0),
                        stop=(ki == 1))
            st = sb_st.tile([128, BLK], F32, tag="st",
                            name=f"st{tagx}{b}{pblk}{rbp}")
            # bias folded into the PSUM->SBUF move (bpt = bp tiled 2x)
            nc.vector.tensor_add(st[:], ps_pj[:], cst["bpt"][:])
            r0 = 128 * (4 * pblk + 2 * rbp)
            dst = out[b, r0:r0 + 256, :].rearrange("(p r) c -> r p c", p=2)
            nc.sync.dma_start(dst, st.rearrange("r (p c) -> r p c", p=2))

    pending = []

    def emit_sums_o(b, blk, sg, pts):
        s = S[b]
        v_sb, oT_sb = s["v"], s["oT"]
        ps_sum = ps_att.tile([128, BLK], F32, tag="att",
                             name=f"pssum{b}{blk}{sg}")
        ps_o = ps_att.tile([128, BLK], F32, tag="att",
                           name=f"pso{b}{blk}{sg}")
        for ko in range(2):
            for hl in range(4):
                nc.tensor.matmul(
                    ps_sum[32 * hl:32 * hl + 32, :],
                    _r(cst["ones32"][:]),
                    _r(pts[hl][:, BLK * ko:BLK * (ko + 1)]),
                    start=(ko == 0), stop=(ko == 1),
                    tile_position=(0, 32 * hl),
                    skip_group_check=True,
                )
        for ko in range(2):
            for hl in range(4):
                hh = 4 * sg + hl
                nc.tensor.matmul(
                    ps_o[32 * hl:32 * hl + 32, :],
                    _r(v_sb[ko][:, 32 * hh:32 * hh + 32]),
                    _r(pts[hl][:, BLK * ko:BLK * (ko + 1)]),
                    start=(ko == 0), stop=(ko == 1),
                    tile_position=(0, 32 * hl),
                    skip_group_check=True,
                )
        R_t = sb_R.tile([128, BLK], F32, tag="R", name=f"R{b}{blk}{sg}")
        nc.vector.reciprocal_approx_fast(R_t[:], ps_sum[:])
        nc.vector.tensor_mul(oT_sb[sg][:, BLK * blk:BLK * (blk + 1)],
                             ps_o[:], R_t[:])

    def flush_pending():
        while pending:
            pending.pop(0)()

    def emit_block(b, blk):
        s = S[b]
        if "oT" not in s:
            s["oT"] = [sb_oT.tile([128, N], BF16, tag=f"oT{k}", bufs=2,
                                  name=f"oT{b}{k}") for k in range(2)]
        kT_sb, qT_sb = s["kT"], s["qT"]
        for sg in range(2):
            pts = []
            for hl in range(4):
                hh = 4 * sg + hl
                st_t = ps_s.tile([128, 2 * BLK], F32, tag="s",
                                 name=f"psst{b}{blk}{hh}")
                for ko in range(2):
                    nc.tensor.matmul(
                        st_t[:, BLK * ko:BLK * (ko + 1)],
                        _r(kT_sb[sg][32 * hl:32 * hl + 32,
                                     128 * ko:128 * (ko + 1)]),
                        _r(qT_sb[sg][32 * hl:32 * hl + 32,
                                     BLK * blk:BLK * (blk + 1)]),
                        start=True, stop=True,
                        tile_position=(32 * hl, 0),
                    )
                pt = sb_pT.tile([128, 2 * BLK], BF16, tag="pT",
                                name=f"pt{b}{blk}{hh}")
                nc.scalar.activation(pt[:], st_t[:], AF.Exp)
                pts.append(pt)
            emit_sums_o(b, blk, sg, pts)
        if blk >= 1:
            emit_proj(b, blk - 1, "m")


    # ---------- emission schedule ----------
    startup_b0()
    if B_LOC > 1:
        chunk_load_x(1)  # prefetch batch-1 xT during batch-0 warm-up
    chunk_kv(0)
    A_CHUNKS[0] = lambda b: None  # in-loop load_x already prefetched
    for b in range(B_LOC):
        for blk in range(NBLK):
            emit_block(b, blk)
            if b + 1 < B_LOC and blk < len(A_CHUNKS):
                A_CHUNKS[blk](b + 1)
        emit_proj(b, NBLK - 1, "t")


def build(with_bp):
    nc = bacc.Bacc("TRN2", target_bir_lowering=False, debug=False,
                   enable_asserts=True)

    def din(name, shape, dtype=F32):
        return nc.dram_tensor(name, shape, dtype, kind="ExternalInput").ap()

    ins = [
        din("xT", [B_LOC, 128, 2 * N], BF16),
        din("wqkvp", [128, 2048], BF16),
        din("srw", [128, 8192], BF16),
        din("f32p", [128, 1154]),
    ]
    out = nc.dram_tensor("out", [B_LOC, N, C], F32, kind="ExternalOutput").ap()

    with tile.TileContext(nc) as tc:
        with ExitStack() as ctx:
            _kernel_body(ctx, tc, out, ins, with_bp)
    nc.compile()
    return nc


def host_prep(inputs):
    """Shared (non-x) host-side tensors, from the full input dict."""
    Wq = np.asarray(inputs["Wq"], np.float32)
    Wkv = np.asarray(inputs["Wkv"], np.float32)
    sr_w = np.asarray(inputs["sr_w"], np.float32)
    sr_b = np.asarray(inputs["sr_b"], np.float32)
    ln_g = np.asarray(inputs["ln_g"], np.float32)
    ln_b = np.asarray(inputs["ln_b"], np.float32)
    Wp = np.asarray(inputs["Wp"], np.float32)
    bp = np.asarray(inputs["bp"], np.float32)

    bf = ml_dtypes.bfloat16
    wq = (Wq * (HD ** -0.5)).astype(np.float32)
    wk = (ln_g[:, None] * Wkv[:, :C]).astype(np.float32)
    wv = (ln_g[:, None] * Wkv[:, C:]).astype(np.float32)
    bias_kv = (ln_b @ Wkv).astype(np.float32)
    srwT = sr_w.transpose(2, 3, 1, 0).reshape(16, C, C)
    # srw[p, 512*tap + 256*ki + o] = srwT[tap, ki*128 + p, o]
    srw8 = np.ascontiguousarray(
        srwT.reshape(16, 2, 128, C).transpose(2, 0, 1, 3)
        .reshape(128, 16 * 2 * C)).astype(bf)

    # packed bf16 weight blob: wq0 wq1 wk0 wk1 wv0 wv1 wp0 wp1
    slabs = []
    for W in (wq, wk, wv, Wp):
        slabs += [W[0:128, :], W[128:256, :]]
    wqkvp = np.concatenate(slabs, axis=1).astype(bf)

    # packed f32 blob: srb(256) bv(256) bpt(512) eye(128) bk(2)
    f32p = np.zeros((128, 1154), np.float32)
    f32p[:, 0:256] = np.broadcast_to(sr_b, (128, C))
    f32p[:, 256:512] = np.broadcast_to(bias_kv[C:], (128, C))
    f32p[:, 512:1024] = np.broadcast_to(np.tile(bp, 2), (128, 2 * C))
    f32p[:, 1024:1152] = np.eye(128, dtype=np.float32)
    f32p[:, 1152] = bias_kv[0:128]
    f32p[:, 1153] = bias_kv[128:256]

    shared = {
        "wqkvp": wqkvp,
        "srw": srw8,
        "f32p": f32p,
    }
    return shared, False


_NC_CACHE = {}


def get_nc(with_bp):
    if with_bp not in _NC_CACHE:
        _NC_CACHE[with_bp] = build(with_bp)
    return _NC_CACHE[with_bp]


def _im2col_perm():
    """idx[tap*256 + m] = spatial row index n for the stride-4 4x4 conv."""
    tap = np.arange(16)
    kh, kw = tap // 4, tap % 4
    m = np.arange(256)
    R, Cc = m // 16, m % 16
    idx = (256 * R[None, :] + 4 * Cc[None, :]
           + 64 * kh[:, None] + kw[:, None])
    return idx.reshape(-1)


IM2COL_IDX = _im2col_perm()


def make_in_maps(inputs):
    x = np.asarray(inputs["x"], np.float32)
    shared, with_bp = host_prep(inputs)
    in_maps = []
    for c in range(N_CORES):
        xc = x[B_LOC * c:B_LOC * (c + 1)]
        xTn = xc.transpose(0, 2, 1)[:, :, IM2COL_IDX]  # [b, C, N]
        xT = np.ascontiguousarray(
            xTn.reshape(B_LOC, 2, 128, N).transpose(0, 2, 1, 3)
            .reshape(B_LOC, 128, 2 * N)).astype(ml_dtypes.bfloat16)
        m = dict(shared)
        m["xT"] = xT
        in_maps.append(m)
    return in_maps, with_bp


def kernel(**inputs):
    in_maps, with_bp = make_in_maps(inputs)
    nc = get_nc(with_bp)
    res = run_bass_kernel_spmd(nc, in_maps, core_ids=list(range(N_CORES)))
    KERNEL_STATS["exec_time_ns"] = res.exec_time_ns
    KERNEL_STATS["mean_exec_time_ns"] = res.mean_exec_time_ns
    KERNEL_STATS["trace"] = res.instructions_and_trace
    out_perm = np.concatenate(
        [res.results[c]["out"] for c in range(N_CORES)], axis=0)
    out = np.empty_like(out_perm)
    out[:, IM2COL_IDX, :] = out_perm
    return out

